# revision 1
# baseline (speedup 1.0000x reference)
"""Trainium2 Bass kernel for nn_APIHyperInputLayer (hypernet MLP, 8-core data parallel).

Math (per branch):
    h   = prelu(F @ W1 + b1, alpha)                       [R, 64]
    w   = (h @ W2 + b2).reshape(R, F, 128)
    hid = einsum('rf,rfo->ro', F, w)
    out = hid.reshape(E, n, 128).sum(1)                   [E, 128]

Restructured: S[k,e,f] = sum_i h[(e,i),k] F[(e,i),f]; out[e,o] =
sum_{k,f} S[k,e,f] W2[k,f*128+o] + (bias term, computed on host).

v2 schedule (all matmuls bf16 -> fp32 PSUM):
  A: hT = blockdiag(W1a,W1e).T @ Fstack  (one weight load, 7 chunk MMs,
     128 out partitions = [ka|ke]); PReLU+bias fused into one scalar-engine
     activation per (chunk, branch, quarter), writing h with 16-row episode
     pitch into h2a/h2e.
  T: 8 XBAR dma transposes (one per branch-quarter) -> row-major h,
     8 episodes per 128-partition group, 32 groups per branch.
  B: per group, two 64-col matmuls sharing one PSUM tile: out parts 0-63 =
     S[k,e,fp] (f first half), parts 64-127 = S[k,e,fp+half] via
     tile_position (0,64).  rhs = block-diag masked features M (el,f cols).
  S->s2: strided engine copies (vector/gpsimd) into s2[kk, fp*256+e] so
     stage C rhs is fully contiguous.
  C: 40 accumulating matmuls out_T[o,e] += W2pair_fp.T @ s2[:, fp block].
Output per core: [128 o, 256 e] fp32; host transposes/concats and adds the
bias term fsum @ b2 (host numpy, exact fp32).
"""

import os
import sys
import functools

import numpy as np

for _p in ("/opt/trn_rl_repo", os.path.expanduser("~/.axon_site/_ro/trn_rl_repo")):
    if os.path.isdir(_p) and _p not in sys.path:
        sys.path.insert(0, _p)

import dataclasses

import ml_dtypes

import concourse.bass as bass
import concourse.bacc as bacc
import concourse.mybir as mybir
import concourse.tile as tile
from concourse.bass_utils import run_bass_kernel_spmd

BF16 = mybir.dt.bfloat16
F32 = mybir.dt.float32

# Problem constants (hardcoded per contest rules)
N_CORES = 8
N_AGENTS, N_ENEMIES = 10, 11
ALLY_F, ENEMY_F = 48, 32
HYPER = 64
OUT = 128
B_FULL = 2048
E_C = B_FULL // N_CORES            # episodes per core = 256
RA = E_C * N_AGENTS                # ally rows per core = 2560
RE = E_C * N_ENEMIES               # enemy rows per core = 2816

PITCH = 16                         # padded rows per episode in h layout
EPG = 8                            # episodes per stage-B group (8*16=128)
NG = E_C // EPG                    # 32 groups per branch
GW_A = EPG * ALLY_F                # 384 M_a cols per group
GW_E = EPG * ENEMY_F               # 256
PAIR_A = ALLY_F // 2               # 24
PAIR_E = ENEMY_F // 2              # 16
MA_FREE = NG * GW_A                # 12288
ME_FREE = NG * GW_E                # 8192
S2A_FREE = PAIR_A * E_C            # 6144
S2E_FREE = PAIR_E * E_C            # 4096
W2COLS = (PAIR_A + PAIR_E) * OUT   # 5120

FS_COLS = RE                       # 2816 fstack cols
CH = 440                           # stage-A chunk cols (44 ally / 40 enemy eps)
CHUNKS = [(c, min(CH, FS_COLS - c)) for c in range(0, FS_COLS, CH)]
A_EPC = CH // N_AGENTS             # 44 ally eps per full chunk
E_EPC = CH // N_ENEMIES            # 40 enemy eps per full chunk
QEP = 64                           # episodes per quarter (1024 padded rows)


def _ap(t, offset, dims):
    """Custom flat AP: dims = [(step, num), ...]; t is an AP or tensor handle."""
    a = t if isinstance(t, bass.AP) else t.ap()
    return dataclasses.replace(a, offset=offset, ap=[[s, n] for (s, n) in dims])


def _prelu_ops(n_per, epc):
    """(chunk, ep0, ep1, quarter) list for one branch, split at quarters."""
    ops = []
    for c in range(len(CHUNKS)):
        e0 = epc * c
        e1 = min(epc * (c + 1), E_C)
        if e0 >= E_C:
            break
        while e0 < e1:
            q = e0 // QEP
            e_mid = min(e1, (q + 1) * QEP)
            ops.append((c, e0, e_mid, q))
            e0 = e_mid
    return ops


def build_program(alpha_a=0.25, alpha_e=0.25):
    nc = bacc.Bacc("TRN2", target_bir_lowering=False, debug=False)

    fstack = nc.declare_dram_parameter("fstack", [80, FS_COLS], BF16, isOutput=False)
    fa = nc.declare_dram_parameter("fa", [RA, ALLY_F], BF16, isOutput=False)
    fe = nc.declare_dram_parameter("fe", [RE, ENEMY_F], BF16, isOutput=False)
    w1blk = nc.declare_dram_parameter("w1blk", [80, 128], BF16, isOutput=False)
    b1cat = nc.declare_dram_parameter("b1cat", [128, 1], F32, isOutput=False)
    w2pack = nc.declare_dram_parameter("w2pack", [128, W2COLS], BF16, isOutput=False)
    out_d = nc.declare_dram_parameter("out", [OUT, E_C], F32, isOutput=True)

    with tile.TileContext(nc) as tc:
        _emit(nc, tc, fstack, fa, fe, w1blk, b1cat, w2pack, out_d, alpha_a, alpha_e)
    nc.compile()
    return nc


def _emit(nc, tc, fstack, fa, fe, w1blk, b1cat, w2pack, out_d, alpha_a, alpha_e):
    from contextlib import ExitStack

    Prelu = mybir.ActivationFunctionType.Prelu

    ctx = ExitStack()
    with ctx:
        const = ctx.enter_context(tc.tile_pool(name="const", bufs=1))
        work = ctx.enter_context(tc.tile_pool(name="work", bufs=1))
        psA = ctx.enter_context(tc.tile_pool(name="psA", bufs=3, space="PSUM"))
        psB = ctx.enter_context(tc.tile_pool(name="psB", bufs=4, space="PSUM"))
        psC = ctx.enter_context(tc.tile_pool(name="psC", bufs=1, space="PSUM"))

        # ---- persistent SBUF ----
        w1_sb = const.tile([80, 128], BF16)
        b1_sb = const.tile([128, 1], F32)
        w2_sb = const.tile([128, W2COLS], BF16)
        fs_sb = const.tile([80, FS_COLS], BF16)
        h2 = [work.tile([128, 1024], BF16, name=f"h2_{q}") for q in range(4)]
        hrA = [work.tile([128, 512], BF16, name=f"hrA_{q}") for q in range(4)]
        hrE = [work.tile([128, 512], BF16, name=f"hrE_{q}") for q in range(4)]
        ma_sb = work.tile([128, MA_FREE], BF16)
        me_sb = work.tile([128, ME_FREE], BF16)
        s2a = work.tile([128, S2A_FREE], BF16)
        s2e = work.tile([128, S2E_FREE], BF16)
        osb = work.tile([OUT, E_C], F32)

        # ---- memsets (vector + gpsimd in parallel) ----
        # h2 dead cols must be finite (they hit M zeros in stage B);
        # M off-diagonal blocks must be exactly zero.
        ma_f32 = ma_sb[:].bitcast(F32)
        me_f32 = me_sb[:].bitcast(F32)
        nc.vector.memset(h2[0][:].bitcast(F32), 0.0)
        nc.gpsimd.memset(h2[1][:].bitcast(F32), 0.0)
        nc.vector.memset(ma_f32[:, 0 : MA_FREE // 4], 0.0)
        nc.gpsimd.memset(ma_f32[:, MA_FREE // 4 : MA_FREE // 2], 0.0)
        nc.vector.memset(h2[2][:].bitcast(F32), 0.0)
        nc.gpsimd.memset(h2[3][:].bitcast(F32), 0.0)
        nc.vector.memset(me_f32[:, 0 : ME_FREE // 4], 0.0)
        nc.gpsimd.memset(me_f32[:, ME_FREE // 4 : ME_FREE // 2], 0.0)

        # ---- parameter loads ----
        # w1/b1 on scalar (needed first, before the prelu chain); fstack as a
        # single DMA on sync.
        nc.scalar.dma_start(w1_sb[:], w1blk.ap())
        nc.scalar.dma_start(b1_sb[:], b1cat.ap())
        nc.sync.dma_start(fs_sb[:], fstack.ap())

        # ---- diagonal DMAs: DRAM features -> block-diagonal M ----
        # One DMA per episode-slot el; partition p = el*PITCH + i.
        def diag_dma(eng, m_sb, f_d, el, n_per, featf, gw, mfree):
            eng.dma_start(
                _ap(m_sb, (el * PITCH) * mfree + el * featf, [
                    (mfree, n_per),          # i: whole partitions
                    (gw, NG),                # group
                    (1, featf),
                ]),
                _ap(f_d, el * n_per * featf, [
                    (featf, n_per),
                    (EPG * n_per * featf, NG),
                    (1, featf),
                ]),
            )

        # ally diag: sync el0-5, gpsimd (SWDGE) el6-7; enemy likewise
        for el in range(EPG):
            eng = nc.sync if el < 6 else nc.gpsimd
            diag_dma(eng, ma_sb, fa, el, N_AGENTS, ALLY_F, GW_A, MA_FREE)
        for el in range(EPG):
            eng = nc.sync if el < 6 else nc.gpsimd
            diag_dma(eng, me_sb, fe, el, N_ENEMIES, ENEMY_F, GW_E, ME_FREE)

        # w2 halves on sync queue (needed only by stage C)
        nc.sync.dma_start(w2_sb[:, 0 : W2COLS // 2], w2pack.ap()[:, 0 : W2COLS // 2])
        nc.sync.dma_start(w2_sb[:, W2COLS // 2 :], w2pack.ap()[:, W2COLS // 2 :])

        # ---- stage A: hT chunks + fused PReLU ----
        pa_tiles = {}
        for ci, (c0, w) in enumerate(CHUNKS):
            pa = psA.tile([128, CH], F32, tag="psA")
            pa_tiles[ci] = pa
            nc.tensor.matmul(
                pa[:, 0:w], w1_sb[:], fs_sb[:, c0 : c0 + w],
                start=True, stop=True,
            )

        ops_a = _prelu_ops(N_AGENTS, A_EPC)
        ops_e = _prelu_ops(N_ENEMIES, E_EPC)
        # order prelu ops by (quarter, chunk); after each quarter's ops,
        # launch that quarter's two XBAR transposes (also on the scalar
        # queue, so they issue as soon as the quarter's h2 is complete).
        prelu_seq = sorted(
            [("a", *op) for op in ops_a] + [("e", *op) for op in ops_e],
            key=lambda t: (t[4], t[1], t[0]))
        done_q = set()

        def launch_transpose(q):
            # q0/q1 issue from the scalar queue (right after their prelus);
            # q2/q3 from sync (after the diag DMAs) to unload scalar.
            eng = nc.scalar if q < 2 else nc.sync
            eng.dma_start(
                _ap(hrA[q], 0, [(512, 128), (64, 8), (1, 64)]),
                h2[q][0:64, :], transpose=True)
            eng.dma_start(
                _ap(hrE[q], 0, [(512, 128), (64, 8), (1, 64)]),
                h2[q][64:128, :], transpose=True)

        for i, (br, c, e0, e1, q) in enumerate(prelu_seq):
            pa = pa_tiles[c]
            n_ep = e1 - e0
            if br == "a":
                n, epc, p0, alpha = N_AGENTS, A_EPC, 0, alpha_a
            else:
                n, epc, p0, alpha = N_ENEMIES, E_EPC, 64, alpha_e
            src = _ap(pa, p0 * CH + (e0 - epc * c) * n,
                      [(CH, 64), (n, n_ep), (1, n)])
            dstap = _ap(h2[q], p0 * 1024 + (e0 - q * QEP) * PITCH,
                        [(1024, 64), (PITCH, n_ep), (1, n)])
            nc.scalar.activation(dstap, src, Prelu,
                                 bias=b1_sb[p0 : p0 + 64, :], scale=1.0,
                                 alpha=alpha)
            if i + 1 == len(prelu_seq) or prelu_seq[i + 1][4] > q:
                if q not in done_q:
                    done_q.add(q)
                    launch_transpose(q)

        # ---- stage B ----
        # Batch gb groups per PSUM tile ([128, 512] f32 = one full bank);
        # one strided reorder copy per tile, vector 2/3 scalar 1/3
        # (gpsimd cannot read PSUM).
        copy_rr = [0]

        def stage_b(hr, m_sb, mfree, featf, pair, gw, s2, s2free, gb):
            half = gw // 2
            for b0 in range(0, NG, gb):
                pb = psB.tile([128, 512], F32, tag="psB")
                for j in range(gb):
                    b = b0 + j
                    q, bb = b // 8, b % 8
                    lhsT = hr[q][:, bb * 64 : bb * 64 + 64]
                    # high half: S[k,e,fp+pair] -> psum parts 64-127
                    nc.tensor.matmul(
                        pb[64:128, j * half : (j + 1) * half], lhsT,
                        _ap(m_sb, b * gw + pair,
                            [(mfree, 128), (featf, EPG), (1, pair)]),
                        start=True, stop=True)
                    # low half: S[k,e,fp] -> psum parts 0-63
                    nc.tensor.matmul(
                        pb[0:64, j * half : (j + 1) * half], lhsT,
                        _ap(m_sb, b * gw,
                            [(mfree, 128), (featf, EPG), (1, pair)]),
                        start=True, stop=True)
                # copy/reorder into s2[kk, fp*E_C + e], e = 8*b0 .. +8*gb
                dst = _ap(s2, b0 * EPG,
                          [(s2free, 128), (E_C, pair), (1, gb * EPG)])
                src = _ap(pb, 0,
                          [(512, 128), (1, pair), (half, gb), (pair, EPG)])
                if copy_rr[0] % 3 == 2:
                    nc.scalar.copy(dst, src)
                else:
                    nc.vector.tensor_copy(dst, src)
                copy_rr[0] += 1

        stage_b(hrA, ma_sb, MA_FREE, ALLY_F, PAIR_A, GW_A, s2a, S2A_FREE, 2)
        stage_b(hrE, me_sb, ME_FREE, ENEMY_F, PAIR_E, GW_E, s2e, S2E_FREE, 4)

        # ---- stage C: out_T[o,e] accumulation over 40 fp slices ----
        pc = psC.tile([OUT, E_C], F32)
        n_sl = PAIR_A + PAIR_E
        idx = 0
        for fp in range(PAIR_A):
            nc.tensor.matmul(
                pc[:], w2_sb[:, fp * OUT : (fp + 1) * OUT],
                s2a[:, fp * E_C : (fp + 1) * E_C],
                start=(idx == 0), stop=(idx == n_sl - 1))
            idx += 1
        for fp in range(PAIR_E):
            nc.tensor.matmul(
                pc[:], w2_sb[:, (PAIR_A + fp) * OUT : (PAIR_A + fp + 1) * OUT],
                s2e[:, fp * E_C : (fp + 1) * E_C],
                start=(idx == 0), stop=(idx == n_sl - 1))
            idx += 1

        nc.vector.tensor_copy(osb[:], pc[:])
        nc.sync.dma_start(out_d.ap(), osb[:])


@functools.lru_cache(maxsize=2)
def _cached_program(alpha_a, alpha_e):
    return build_program(alpha_a, alpha_e)


def host_prep(ally_features, enemy_features, Wa1, ba1, aa, Wa2, ba2,
              We1, be1, ae, We2, be2):
    """Per-core input maps (numpy, bf16) + host-side bias term."""
    bf = ml_dtypes.bfloat16

    def uniform_alpha(a):
        a = np.asarray(a, dtype=np.float32)
        assert np.allclose(a, a[0]), "per-channel alpha not supported"
        return float(a[0])

    ua, ue = uniform_alpha(aa), uniform_alpha(ae)

    w1 = np.zeros((80, 128), dtype=np.float32)
    w1[0:ALLY_F, 0:HYPER] = np.asarray(Wa1)
    w1[ALLY_F:80, HYPER:128] = np.asarray(We1)
    w1 = w1.astype(bf)
    b1 = np.concatenate([np.asarray(ba1), np.asarray(be1)]).astype(np.float32)
    b1 = np.ascontiguousarray(b1.reshape(128, 1))

    w2 = np.zeros((128, W2COLS), dtype=np.float32)
    Wa2_, We2_ = np.asarray(Wa2), np.asarray(We2)
    for fp in range(PAIR_A):
        w2[0:HYPER, fp * OUT : (fp + 1) * OUT] = Wa2_[:, fp * OUT : (fp + 1) * OUT]
        w2[HYPER:128, fp * OUT : (fp + 1) * OUT] = \
            Wa2_[:, (fp + PAIR_A) * OUT : (fp + PAIR_A + 1) * OUT]
    for fp in range(PAIR_E):
        c = (PAIR_A + fp) * OUT
        w2[0:HYPER, c : c + OUT] = We2_[:, fp * OUT : (fp + 1) * OUT]
        w2[HYPER:128, c : c + OUT] = \
            We2_[:, (fp + PAIR_E) * OUT : (fp + PAIR_E + 1) * OUT]
    w2 = w2.astype(bf)

    fa_all = np.asarray(ally_features, dtype=np.float32)
    fe_all = np.asarray(enemy_features, dtype=np.float32)
    fa_bf = fa_all.astype(bf)
    fe_bf = fe_all.astype(bf)

    # host-side bias term: fsum @ b2 (exact fp32)
    fsum_a = fa_all.reshape(B_FULL, N_AGENTS, ALLY_F).sum(axis=1)
    fsum_e = fe_all.reshape(B_FULL, N_ENEMIES, ENEMY_F).sum(axis=1)
    bias_out = (fsum_a @ np.asarray(ba2).reshape(ALLY_F, OUT)
                + fsum_e @ np.asarray(be2).reshape(ENEMY_F, OUT)).astype(np.float32)

    in_maps = []
    for c in range(N_CORES):
        fa_c = np.ascontiguousarray(fa_bf[c * RA : (c + 1) * RA])
        fe_c = np.ascontiguousarray(fe_bf[c * RE : (c + 1) * RE])
        fs = np.zeros((80, FS_COLS), dtype=bf)
        fs[0:ALLY_F, 0:RA] = fa_c.T
        fs[ALLY_F:80, 0:RE] = fe_c.T
        in_maps.append({
            "fstack": np.ascontiguousarray(fs),
            "fa": fa_c, "fe": fe_c,
            "w1blk": w1, "b1cat": b1, "w2pack": w2,
        })
    aux = {"bias_out": bias_out, "ua": ua, "ue": ue}
    return in_maps, aux


def assemble_output(results, aux):
    outs = [np.asarray(r["out"], dtype=np.float32) for r in results]
    dev = np.concatenate([o.T for o in outs], axis=0)
    return dev + aux["bias_out"]


def kernel(**inputs) -> np.ndarray:
    in_maps, aux = host_prep(**inputs)
    nc = _cached_program(aux["ua"], aux["ue"])
    res = run_bass_kernel_spmd(nc, in_maps, core_ids=list(range(N_CORES)))
    return assemble_output(res.results, aux)


if __name__ == "__main__":
    build_program()
    print("built ok")



# revision 7
# speedup vs baseline: 1.2274x; 1.2274x over previous
"""Trainium2 Bass kernel for nn_APIHyperInputLayer (hypernet MLP, 8-core data parallel).

Math (per branch):
    h   = prelu(F @ W1 + b1, alpha)                       [R, 64]
    w   = (h @ W2 + b2).reshape(R, F, 128)
    hid = einsum('rf,rfo->ro', F, w)
    out = hid.reshape(E, n, 128).sum(1)                   [E, 128]

Restructured: S[k,e,f] = sum_i h[(e,i),k] F[(e,i),f]; out[e,o] =
sum_{k,f} S[k,e,f] W2[k,f*128+o] + (bias term, computed on host).

v3 schedule (row-major h; no transposes):
  Episodes padded to PITCH=16 rows; 8 episodes = one 128-partition group.
  A: per group g, matmul(out=[128 rows, 128 k], lhsT=fsp[:, g*128:+128],
     rhs=w1ext) where fsp = [81, 4096] padded-transposed features with a
     ones-row; w1ext = block-diag(Wa1, We1) with bias row.  h comes out
     row-major directly; PReLU (pure, alpha=0.25) evacuates PSUM->SBUF
     once per 8 groups.
  B: per group, two 64-part matmuls sharing one PSUM tile (f-pair lo/hi in
     partition halves); rhs = block-diag masked features M built by one
     4-dim-AP DMA per branch from host-padded row-major features.
  evac: strided casts PSUM->s2[kk, fp*256+e] (vector 2/3, scalar 1/3).
  C: 40 accumulating matmuls out_T[o,e] += W2pair_fp.T @ s2 slice.
Output per core: [128 o, 256 e] fp32; host transposes/concats and adds the
bias term fsum @ b2 (host numpy, exact fp32).
"""

import os
import sys
import functools

import numpy as np

for _p in ("/opt/trn_rl_repo", os.path.expanduser("~/.axon_site/_ro/trn_rl_repo")):
    if os.path.isdir(_p) and _p not in sys.path:
        sys.path.insert(0, _p)

import dataclasses

import ml_dtypes

import concourse.bass as bass
import concourse.bacc as bacc
import concourse.mybir as mybir
import concourse.tile as tile
from concourse.bass_utils import run_bass_kernel_spmd

BF16 = mybir.dt.bfloat16
F32 = mybir.dt.float32

# Problem constants (hardcoded per contest rules)
N_CORES = 8
N_AGENTS, N_ENEMIES = 10, 11
ALLY_F, ENEMY_F = 48, 32
HYPER = 64
OUT = 128
B_FULL = 2048
E_C = B_FULL // N_CORES            # episodes per core = 256

PITCH = 16                         # padded rows per episode
EPG = 8                            # episodes per group (8*16=128 partitions)
NG = E_C // EPG                    # 32 groups
PROWS = E_C * PITCH                # padded rows per core = 4096
GW_A = EPG * ALLY_F                # 384 M_a cols per group
GW_E = EPG * ENEMY_F               # 256
PAIR_A = ALLY_F // 2               # 24
PAIR_E = ENEMY_F // 2              # 16
MA_FREE = NG * GW_A                # 12288
ME_FREE = NG * GW_E                # 8192
S2A_FREE = PAIR_A * E_C            # 6144
S2E_FREE = PAIR_E * E_C            # 4096
W2COLS = (PAIR_A + PAIR_E) * OUT   # 5120
KROWS = ALLY_F + ENEMY_F + 1       # 81 = stacked features + ones row

BATCH = 8                          # groups per prelu batch (8*128 = 1024 cols)
NBATCH = NG // BATCH               # 4
IMG_G = 8                          # leading groups delivered as dense host image
IMA_COLS = IMG_G * GW_A            # 3072
IME_COLS = IMG_G * GW_E            # 2048


def _ap(t, offset, dims):
    """Custom flat AP: dims = [(step, num), ...]; t is an AP or tensor handle."""
    a = t if isinstance(t, bass.AP) else t.ap()
    return dataclasses.replace(a, offset=offset, ap=[[s, n] for (s, n) in dims])


def build_program(alpha_a=0.25, alpha_e=0.25):
    assert alpha_a == alpha_e, "branches must share alpha"
    nc = bacc.Bacc("TRN2", target_bir_lowering=False, debug=False)

    fsp = nc.declare_dram_parameter("fsp", [KROWS, PROWS], BF16, isOutput=False)
    fap = nc.declare_dram_parameter("fap", [PROWS, ALLY_F], BF16, isOutput=False)
    fep = nc.declare_dram_parameter("fep", [PROWS, ENEMY_F], BF16, isOutput=False)
    ma_img = nc.declare_dram_parameter("ma_img", [128, IMA_COLS], BF16, isOutput=False)
    me_img = nc.declare_dram_parameter("me_img", [128, IME_COLS], BF16, isOutput=False)
    w1e = nc.declare_dram_parameter("w1e", [KROWS, 128], BF16, isOutput=False)
    w2pack = nc.declare_dram_parameter("w2pack", [128, W2COLS], BF16, isOutput=False)
    out_d = nc.declare_dram_parameter("out", [OUT, E_C], F32, isOutput=True)

    with tile.TileContext(nc) as tc:
        _emit(nc, tc, fsp, fap, fep, ma_img, me_img, w1e, w2pack, out_d, alpha_a)
    nc.compile()
    return nc


def _emit(nc, tc, fsp, fap, fep, ma_img, me_img, w1e, w2pack, out_d, alpha):
    from contextlib import ExitStack

    Prelu = mybir.ActivationFunctionType.Prelu

    ctx = ExitStack()
    with ctx:
        const = ctx.enter_context(tc.tile_pool(name="const", bufs=1))
        work = ctx.enter_context(tc.tile_pool(name="work", bufs=1))
        psA = ctx.enter_context(tc.tile_pool(name="psA", bufs=2, space="PSUM"))
        psB = ctx.enter_context(tc.tile_pool(name="psB", bufs=3, space="PSUM"))
        psC = ctx.enter_context(tc.tile_pool(name="psC", bufs=1, space="PSUM"))

        # ---- persistent SBUF ----
        fsp_sb = const.tile([KROWS, PROWS], BF16)
        w1_sb = const.tile([KROWS, 128], BF16)
        w2_sb = const.tile([128, W2COLS], BF16)
        h_sb = work.tile([128, PROWS], BF16)
        ma_sb = work.tile([128, MA_FREE], BF16)
        me_sb = work.tile([128, ME_FREE], BF16)
        s2a = work.tile([128, S2A_FREE], BF16)
        s2e = work.tile([128, S2E_FREE], BF16)
        osb = work.tile([OUT, E_C], F32)

        # ---- memsets for M off-diagonal zeros, groups IMG_G..NG only ----
        # (the leading IMG_G groups arrive as a dense host image with zeros)
        ma_f32 = ma_sb[:].bitcast(F32)
        me_f32 = me_sb[:].bitcast(F32)
        a0, a1 = IMA_COLS // 2, MA_FREE // 2        # f32 col indices
        e0_, e1_ = IME_COLS // 2, ME_FREE // 2
        nc.vector.memset(ma_f32[:, a0 : (a0 + a1) // 2], 0.0)
        nc.gpsimd.memset(ma_f32[:, (a0 + a1) // 2 : a1], 0.0)
        nc.vector.memset(me_f32[:, e0_ : (e0_ + e1_) // 2], 0.0)
        nc.gpsimd.memset(me_f32[:, (e0_ + e1_) // 2 : e1_], 0.0)

        # ---- parameter loads ----
        # sync: stage-A/B critical loads first.
        CHW = PROWS // NBATCH      # 1024 cols per chunk = one batch
        nc.sync.dma_start(fsp_sb[:, 0:CHW], fsp.ap()[:, 0:CHW])
        nc.sync.dma_start(ma_sb[:, 0:IMA_COLS], ma_img.ap())
        nc.sync.dma_start(me_sb[:, 0:IME_COLS], me_img.ap())
        for c in range(1, NBATCH):
            nc.sync.dma_start(
                fsp_sb[:, c * CHW : (c + 1) * CHW],
                fsp.ap()[:, c * CHW : (c + 1) * CHW])

        # scalar: w1 first; w2 halves are interleaved between prelus below.
        nc.scalar.dma_start(w1_sb[:], w1e.ap())

        # M diag loads for groups IMG_G..NG: one DMA per (branch, el),
        # spread over sync + gpsimd.  dst partition (el*16+i), col
        # g*gw + el*featf + f; src = padded row-major features (pad rows
        # zero, so i in 0..16 also writes the in-diagonal zeros).
        NGR = NG - IMG_G
        def el_dma(eng, el, m_sb, f_d, featf, gw, mfree):
            eng.dma_start(
                _ap(m_sb, el * PITCH * mfree + IMG_G * gw + el * featf, [
                    (mfree, PITCH),                 # i: partitions
                    (gw, NGR),                      # group
                    (1, featf),
                ]),
                _ap(f_d, (IMG_G * 128 + el * PITCH) * featf, [
                    (featf, PITCH),
                    (128 * featf, NGR),
                    (1, featf),
                ]),
            )

        for el in range(EPG):
            eng = nc.sync if el < 4 else nc.gpsimd
            el_dma(eng, el, ma_sb, fap, ALLY_F, GW_A, MA_FREE)
            el_dma(eng, el, me_sb, fep, ENEMY_F, GW_E, ME_FREE)

        # ---- pipeline: per batch of 8 groups: A matmuls, prelu, B matmuls,
        # evac casts ----
        copy_rr = [0]

        def evac(dst, src):
            if copy_rr[0] % 3 == 2:
                nc.scalar.copy(dst, src)
            else:
                nc.vector.tensor_copy(dst, src)
            copy_rr[0] += 1

        for b in range(NBATCH):
            # stage A: h rows for groups 8b..8b+7
            pa = psA.tile([128, BATCH * 128], F32, tag="psA")
            for j in range(BATCH):
                g = b * BATCH + j
                nc.tensor.matmul(
                    pa[:, j * 128 : (j + 1) * 128],
                    fsp_sb[:, g * 128 : (g + 1) * 128],
                    w1_sb[:],
                    start=True, stop=True)
            nc.scalar.activation(
                h_sb[:, b * 1024 : (b + 1) * 1024], pa[:],
                Prelu, scale=1.0, alpha=alpha)
            if b < 2:
                # w2 halves ride the scalar queue between prelus (needed
                # only by stage C, but must not sit behind the el DMAs)
                nc.scalar.dma_start(
                    w2_sb[:, b * (W2COLS // 2) : (b + 1) * (W2COLS // 2)],
                    w2pack.ap()[:, b * (W2COLS // 2) : (b + 1) * (W2COLS // 2)])

            # stage B ally: 2 groups per PSUM tile
            for j2 in range(BATCH // 2):
                g0 = b * BATCH + 2 * j2
                pb = psB.tile([128, 512], F32, tag="psB")
                for u in range(2):
                    g = g0 + u
                    lhsT = h_sb[:, g * 128 : g * 128 + 64]
                    nc.tensor.matmul(
                        pb[64:128, u * 192 : (u + 1) * 192], lhsT,
                        _ap(ma_sb, g * GW_A + PAIR_A,
                            [(MA_FREE, 128), (ALLY_F, EPG), (1, PAIR_A)]),
                        start=True, stop=True)
                    nc.tensor.matmul(
                        pb[0:64, u * 192 : (u + 1) * 192], lhsT,
                        _ap(ma_sb, g * GW_A,
                            [(MA_FREE, 128), (ALLY_F, EPG), (1, PAIR_A)]),
                        start=True, stop=True)
                evac(
                    _ap(s2a, g0 * EPG,
                        [(S2A_FREE, 128), (E_C, PAIR_A), (1, 2 * EPG)]),
                    _ap(pb, 0,
                        [(512, 128), (1, PAIR_A), (192, 2), (PAIR_A, EPG)]))

            # stage B enemy: 4 groups per PSUM tile
            for j3 in range(BATCH // 4):
                g0 = b * BATCH + 4 * j3
                pb = psB.tile([128, 512], F32, tag="psB")
                for u in range(4):
                    g = g0 + u
                    lhsT = h_sb[:, g * 128 + 64 : g * 128 + 128]
                    nc.tensor.matmul(
                        pb[64:128, u * 128 : (u + 1) * 128], lhsT,
                        _ap(me_sb, g * GW_E + PAIR_E,
                            [(ME_FREE, 128), (ENEMY_F, EPG), (1, PAIR_E)]),
                        start=True, stop=True)
                    nc.tensor.matmul(
                        pb[0:64, u * 128 : (u + 1) * 128], lhsT,
                        _ap(me_sb, g * GW_E,
                            [(ME_FREE, 128), (ENEMY_F, EPG), (1, PAIR_E)]),
                        start=True, stop=True)
                evac(
                    _ap(s2e, g0 * EPG,
                        [(S2E_FREE, 128), (E_C, PAIR_E), (1, 4 * EPG)]),
                    _ap(pb, 0,
                        [(512, 128), (1, PAIR_E), (128, 4), (PAIR_E, EPG)]))

        # ---- stage C: out_T[o,e] accumulation over 40 fp slices ----
        pc = psC.tile([OUT, E_C], F32)
        n_sl = PAIR_A + PAIR_E
        idx = 0
        for fp in range(PAIR_A):
            nc.tensor.matmul(
                pc[:], w2_sb[:, fp * OUT : (fp + 1) * OUT],
                s2a[:, fp * E_C : (fp + 1) * E_C],
                start=(idx == 0), stop=(idx == n_sl - 1))
            idx += 1
        for fp in range(PAIR_E):
            nc.tensor.matmul(
                pc[:], w2_sb[:, (PAIR_A + fp) * OUT : (PAIR_A + fp + 1) * OUT],
                s2e[:, fp * E_C : (fp + 1) * E_C],
                start=(idx == 0), stop=(idx == n_sl - 1))
            idx += 1

        nc.vector.tensor_copy(osb[:], pc[:])
        nc.sync.dma_start(out_d.ap(), osb[:])


@functools.lru_cache(maxsize=2)
def _cached_program(alpha_a, alpha_e):
    return build_program(alpha_a, alpha_e)


def host_prep(ally_features, enemy_features, Wa1, ba1, aa, Wa2, ba2,
              We1, be1, ae, We2, be2):
    """Per-core input maps (numpy, bf16) + host-side bias term."""
    bf = ml_dtypes.bfloat16

    def uniform_alpha(a):
        a = np.asarray(a, dtype=np.float32)
        assert np.allclose(a, a[0]), "per-channel alpha not supported"
        return float(a[0])

    ua, ue = uniform_alpha(aa), uniform_alpha(ae)

    w1 = np.zeros((KROWS, 128), dtype=np.float32)
    w1[0:ALLY_F, 0:HYPER] = np.asarray(Wa1)
    w1[ALLY_F:80, HYPER:128] = np.asarray(We1)
    w1[80, 0:HYPER] = np.asarray(ba1)
    w1[80, HYPER:128] = np.asarray(be1)
    w1 = w1.astype(bf)

    w2 = np.zeros((128, W2COLS), dtype=np.float32)
    Wa2_, We2_ = np.asarray(Wa2), np.asarray(We2)
    for fp in range(PAIR_A):
        w2[0:HYPER, fp * OUT : (fp + 1) * OUT] = Wa2_[:, fp * OUT : (fp + 1) * OUT]
        w2[HYPER:128, fp * OUT : (fp + 1) * OUT] = \
            Wa2_[:, (fp + PAIR_A) * OUT : (fp + PAIR_A + 1) * OUT]
    for fp in range(PAIR_E):
        c = (PAIR_A + fp) * OUT
        w2[0:HYPER, c : c + OUT] = We2_[:, fp * OUT : (fp + 1) * OUT]
        w2[HYPER:128, c : c + OUT] = \
            We2_[:, (fp + PAIR_E) * OUT : (fp + PAIR_E + 1) * OUT]
    w2 = w2.astype(bf)

    fa_all = np.asarray(ally_features, dtype=np.float32)
    fe_all = np.asarray(enemy_features, dtype=np.float32)
    fa_bf = fa_all.astype(bf)
    fe_bf = fe_all.astype(bf)

    # host-side bias term: fsum @ b2 (exact fp32)
    fsum_a = fa_all.reshape(B_FULL, N_AGENTS, ALLY_F).sum(axis=1)
    fsum_e = fe_all.reshape(B_FULL, N_ENEMIES, ENEMY_F).sum(axis=1)
    bias_out = (fsum_a @ np.asarray(ba2).reshape(ALLY_F, OUT)
                + fsum_e @ np.asarray(be2).reshape(ENEMY_F, OUT)).astype(np.float32)

    RA = E_C * N_AGENTS
    RE = E_C * N_ENEMIES
    in_maps = []
    for c in range(N_CORES):
        fa_c = fa_bf[c * RA : (c + 1) * RA].reshape(E_C, N_AGENTS, ALLY_F)
        fe_c = fe_bf[c * RE : (c + 1) * RE].reshape(E_C, N_ENEMIES, ENEMY_F)
        # padded row-major features [E_C*16, featf], pad rows zero
        fa_pad = np.zeros((E_C, PITCH, ALLY_F), dtype=bf)
        fa_pad[:, :N_AGENTS, :] = fa_c
        fe_pad = np.zeros((E_C, PITCH, ENEMY_F), dtype=bf)
        fe_pad[:, :N_ENEMIES, :] = fe_c
        fa_pad = fa_pad.reshape(PROWS, ALLY_F)
        fe_pad = fe_pad.reshape(PROWS, ENEMY_F)
        # fsp: [81, PROWS] = [ally F^T padded; enemy F^T padded; ones]
        fs = np.zeros((KROWS, PROWS), dtype=bf)
        fs[0:ALLY_F, :] = fa_pad.T
        fs[ALLY_F:80, :] = fe_pad.T
        fs[80, :] = np.float32(1.0)

        # dense block-diag M image for the first IMG_G groups
        def m_image(f_pad, featf, gw, cols):
            img = np.zeros((EPG, PITCH, IMG_G, EPG, featf), dtype=bf)
            f4 = f_pad[: IMG_G * 128].reshape(IMG_G, EPG, PITCH, featf)
            for el in range(EPG):
                img[el, :, :, el, :] = f4[:, el].transpose(1, 0, 2)
            return np.ascontiguousarray(img.reshape(128, cols))

        in_maps.append({
            "fsp": np.ascontiguousarray(fs),
            "fap": np.ascontiguousarray(fa_pad),
            "fep": np.ascontiguousarray(fe_pad),
            "ma_img": m_image(fa_pad, ALLY_F, GW_A, IMA_COLS),
            "me_img": m_image(fe_pad, ENEMY_F, GW_E, IME_COLS),
            "w1e": w1, "w2pack": w2,
        })
    aux = {"bias_out": bias_out, "ua": ua, "ue": ue}
    return in_maps, aux


def assemble_output(results, aux):
    outs = [np.asarray(r["out"], dtype=np.float32) for r in results]
    dev = np.concatenate([o.T for o in outs], axis=0)
    return dev + aux["bias_out"]


def kernel(**inputs) -> np.ndarray:
    in_maps, aux = host_prep(**inputs)
    nc = _cached_program(aux["ua"], aux["ue"])
    res = run_bass_kernel_spmd(nc, in_maps, core_ids=list(range(N_CORES)))
    return assemble_output(res.results, aux)


if __name__ == "__main__":
    build_program()
    print("built ok")


# revision 10
# speedup vs baseline: 1.2409x; 1.0110x over previous
"""Trainium2 Bass kernel for nn_APIHyperInputLayer (hypernet MLP, 8-core data parallel).

Math (per branch):
    h   = prelu(F @ W1 + b1, alpha)                       [R, 64]
    w   = (h @ W2 + b2).reshape(R, F, 128)
    hid = einsum('rf,rfo->ro', F, w)
    out = hid.reshape(E, n, 128).sum(1)                   [E, 128]

Restructured: S[k,e,f] = sum_i h[(e,i),k] F[(e,i),f]; out[e,o] =
sum_{k,f} S[k,e,f] W2[k,f*128+o] + (bias term, computed on host).

v3 schedule (row-major h; no transposes):
  Episodes padded to PITCH=16 rows; 8 episodes = one 128-partition group.
  A: per group g, matmul(out=[128 rows, 128 k], lhsT=fsp[:, g*128:+128],
     rhs=w1ext) where fsp = [81, 4096] padded-transposed features with a
     ones-row; w1ext = block-diag(Wa1, We1) with bias row.  h comes out
     row-major directly; PReLU (pure, alpha=0.25) evacuates PSUM->SBUF
     once per 8 groups.
  B: per group, two 64-part matmuls sharing one PSUM tile (f-pair lo/hi in
     partition halves); rhs = block-diag masked features M built by one
     4-dim-AP DMA per branch from host-padded row-major features.
  evac: strided casts PSUM->s2[kk, fp*256+e] (vector 2/3, scalar 1/3).
  C: 40 accumulating matmuls out_T[o,e] += W2pair_fp.T @ s2 slice.
Output per core: [128 o, 256 e] fp32; host transposes/concats and adds the
bias term fsum @ b2 (host numpy, exact fp32).
"""

import os
import sys
import functools

import numpy as np

for _p in ("/opt/trn_rl_repo", os.path.expanduser("~/.axon_site/_ro/trn_rl_repo")):
    if os.path.isdir(_p) and _p not in sys.path:
        sys.path.insert(0, _p)

import dataclasses

import ml_dtypes

import concourse.bass as bass
import concourse.bacc as bacc
import concourse.mybir as mybir
import concourse.tile as tile
from concourse.bass_utils import run_bass_kernel_spmd

BF16 = mybir.dt.bfloat16
F32 = mybir.dt.float32

# Problem constants (hardcoded per contest rules)
N_CORES = 8
N_AGENTS, N_ENEMIES = 10, 11
ALLY_F, ENEMY_F = 48, 32
HYPER = 64
OUT = 128
B_FULL = 2048
E_C = B_FULL // N_CORES            # episodes per core = 256

PITCH = 16                         # padded rows per episode
EPG = 8                            # episodes per group (8*16=128 partitions)
NG = E_C // EPG                    # 32 groups
PROWS = E_C * PITCH                # padded rows per core = 4096
GW_A = EPG * ALLY_F                # 384 M_a cols per group
GW_E = EPG * ENEMY_F               # 256
PAIR_A = ALLY_F // 2               # 24
PAIR_E = ENEMY_F // 2              # 16
MA_FREE = NG * GW_A                # 12288
ME_FREE = NG * GW_E                # 8192
S2A_FREE = PAIR_A * E_C            # 6144
S2E_FREE = PAIR_E * E_C            # 4096
W2COLS = (PAIR_A + PAIR_E) * OUT   # 5120
KROWS = ALLY_F + ENEMY_F + 1       # 81 = stacked features + ones row

BATCH = 8                          # groups per prelu batch (8*128 = 1024 cols)
NBATCH = NG // BATCH               # 4
IMG_G = 8                          # leading groups delivered as dense host image
IMA_COLS = IMG_G * GW_A            # 3072
IME_COLS = IMG_G * GW_E            # 2048


def _ap(t, offset, dims):
    """Custom flat AP: dims = [(step, num), ...]; t is an AP or tensor handle."""
    a = t if isinstance(t, bass.AP) else t.ap()
    return dataclasses.replace(a, offset=offset, ap=[[s, n] for (s, n) in dims])


def build_program(alpha_a=0.25, alpha_e=0.25):
    assert alpha_a == alpha_e, "branches must share alpha"
    nc = bacc.Bacc("TRN2", target_bir_lowering=False, debug=False)

    fsp = nc.declare_dram_parameter("fsp", [KROWS, PROWS], BF16, isOutput=False)
    fap = nc.declare_dram_parameter("fap", [PROWS, ALLY_F], BF16, isOutput=False)
    fep = nc.declare_dram_parameter("fep", [PROWS, ENEMY_F], BF16, isOutput=False)
    ma_img = nc.declare_dram_parameter("ma_img", [128, IMA_COLS], BF16, isOutput=False)
    me_img = nc.declare_dram_parameter("me_img", [128, IME_COLS], BF16, isOutput=False)
    w1e = nc.declare_dram_parameter("w1e", [KROWS, 128], BF16, isOutput=False)
    w2pack = nc.declare_dram_parameter("w2pack", [128, W2COLS], BF16, isOutput=False)
    out_d = nc.declare_dram_parameter("out", [OUT, E_C], F32, isOutput=True)

    with tile.TileContext(nc) as tc:
        _emit(nc, tc, fsp, fap, fep, ma_img, me_img, w1e, w2pack, out_d, alpha_a)
    nc.compile()
    return nc


def _emit(nc, tc, fsp, fap, fep, ma_img, me_img, w1e, w2pack, out_d, alpha):
    from contextlib import ExitStack

    Prelu = mybir.ActivationFunctionType.Prelu

    ctx = ExitStack()
    with ctx:
        const = ctx.enter_context(tc.tile_pool(name="const", bufs=1))
        work = ctx.enter_context(tc.tile_pool(name="work", bufs=1))
        psA = ctx.enter_context(tc.tile_pool(name="psA", bufs=3, space="PSUM"))
        psB = ctx.enter_context(tc.tile_pool(name="psB", bufs=3, space="PSUM"))
        psC = ctx.enter_context(tc.tile_pool(name="psC", bufs=1, space="PSUM"))
        psW = ctx.enter_context(tc.tile_pool(name="psW", bufs=1, space="PSUM"))

        # ---- persistent SBUF ----
        fsp_sb = const.tile([KROWS, PROWS], BF16)
        w1_sb = const.tile([KROWS, 128], BF16)
        w2_sb = const.tile([128, W2COLS], BF16)
        h_sb = work.tile([128, PROWS], BF16)
        ma_sb = work.tile([128, MA_FREE], BF16)
        me_sb = work.tile([128, ME_FREE], BF16)
        s2a = work.tile([128, S2A_FREE], BF16)
        s2e = work.tile([128, S2E_FREE], BF16)
        osb = work.tile([OUT, E_C], F32)

        # ---- PE warmup: junk matmuls to flip the HAM clock gate to 8/8
        # before real work arrives (reads a memset junk tile) ----
        junk = work.tile([128, 64], BF16)
        nc.vector.memset(junk[:].bitcast(F32), 0.0)
        pw = psW.tile([128, 512], F32)
        for wi in range(18):
            nc.tensor.matmul(pw[0:64, 0:64], junk[:], junk[:],
                             start=True, stop=True)

        # ---- memsets for M off-diagonal zeros, groups IMG_G..NG only ----
        # (the leading IMG_G groups arrive as a dense host image with zeros)
        ma_f32 = ma_sb[:].bitcast(F32)
        me_f32 = me_sb[:].bitcast(F32)
        a0, a1 = IMA_COLS // 2, MA_FREE // 2        # f32 col indices
        e0_, e1_ = IME_COLS // 2, ME_FREE // 2
        nc.vector.memset(ma_f32[:, a0 : (a0 + a1) // 2], 0.0)
        nc.gpsimd.memset(ma_f32[:, (a0 + a1) // 2 : a1], 0.0)
        nc.vector.memset(me_f32[:, e0_ : (e0_ + e1_) // 2], 0.0)
        nc.gpsimd.memset(me_f32[:, (e0_ + e1_) // 2 : e1_], 0.0)

        # ---- parameter loads ----
        # sync ring: fsp chunk 0 + M images (stage-B batch-0 critical path);
        # scalar ring: w1 + remaining fsp chunks (stage-A path).
        CHW = PROWS // NBATCH      # 1024 cols per chunk = one batch
        nc.sync.dma_start(fsp_sb[:, 0:CHW], fsp.ap()[:, 0:CHW])
        nc.sync.dma_start(ma_sb[:, 0:IMA_COLS], ma_img.ap())
        nc.sync.dma_start(me_sb[:, 0:IME_COLS], me_img.ap())
        nc.scalar.dma_start(w1_sb[:], w1e.ap())
        for c in range(1, NBATCH):
            nc.scalar.dma_start(
                fsp_sb[:, c * CHW : (c + 1) * CHW],
                fsp.ap()[:, c * CHW : (c + 1) * CHW])

        # M diag loads for groups IMG_G..NG: one DMA per (branch, el),
        # spread over sync + gpsimd.  dst partition (el*16+i), col
        # g*gw + el*featf + f; src = padded row-major features (pad rows
        # zero, so i in 0..16 also writes the in-diagonal zeros).
        NGR = NG - IMG_G
        def el_dma(eng, el, m_sb, f_d, featf, gw, mfree):
            eng.dma_start(
                _ap(m_sb, el * PITCH * mfree + IMG_G * gw + el * featf, [
                    (mfree, PITCH),                 # i: partitions
                    (gw, NGR),                      # group
                    (1, featf),
                ]),
                _ap(f_d, (IMG_G * 128 + el * PITCH) * featf, [
                    (featf, PITCH),
                    (128 * featf, NGR),
                    (1, featf),
                ]),
            )

        for el in range(EPG):
            eng = nc.sync if el < 4 else nc.gpsimd
            el_dma(eng, el, ma_sb, fap, ALLY_F, GW_A, MA_FREE)
        for el in range(EPG):
            eng = nc.sync if el < 4 else nc.gpsimd
            el_dma(eng, el, me_sb, fep, ENEMY_F, GW_E, ME_FREE)

        # ---- pipeline: per sub-batch of 4 groups: A matmuls + prelu;
        # per batch of 8 groups: B matmuls + evac casts ----
        copy_rr = [0]

        def evac(dst, src):
            if copy_rr[0] % 3 == 2:
                nc.scalar.copy(dst, src)
            else:
                nc.vector.tensor_copy(dst, src)
            copy_rr[0] += 1

        SUBG = 4                     # groups per psA tile / prelu

        def a_stage(sub):
            pa = psA.tile([128, SUBG * 128], F32, tag="psA")
            for j in range(SUBG):
                g = sub * SUBG + j
                nc.tensor.matmul(
                    pa[:, j * 128 : (j + 1) * 128],
                    fsp_sb[:, g * 128 : (g + 1) * 128],
                    w1_sb[:],
                    start=True, stop=True)
            nc.scalar.activation(
                h_sb[:, sub * 512 : (sub + 1) * 512], pa[:],
                Prelu, scale=1.0, alpha=alpha)

        def b_ally(g0):              # 2 groups per PSUM tile
            pb = psB.tile([128, 512], F32, tag="psB")
            for u in range(2):
                g = g0 + u
                lhsT = h_sb[:, g * 128 : g * 128 + 64]
                nc.tensor.matmul(
                    pb[64:128, u * 192 : (u + 1) * 192], lhsT,
                    _ap(ma_sb, g * GW_A + PAIR_A,
                        [(MA_FREE, 128), (ALLY_F, EPG), (1, PAIR_A)]),
                    start=True, stop=True)
                nc.tensor.matmul(
                    pb[0:64, u * 192 : (u + 1) * 192], lhsT,
                    _ap(ma_sb, g * GW_A,
                        [(MA_FREE, 128), (ALLY_F, EPG), (1, PAIR_A)]),
                    start=True, stop=True)
            evac(
                _ap(s2a, g0 * EPG,
                    [(S2A_FREE, 128), (E_C, PAIR_A), (1, 2 * EPG)]),
                _ap(pb, 0,
                    [(512, 128), (1, PAIR_A), (192, 2), (PAIR_A, EPG)]))

        def b_enemy(g0):             # 4 groups per PSUM tile
            pb = psB.tile([128, 512], F32, tag="psB")
            for u in range(4):
                g = g0 + u
                lhsT = h_sb[:, g * 128 + 64 : g * 128 + 128]
                nc.tensor.matmul(
                    pb[64:128, u * 128 : (u + 1) * 128], lhsT,
                    _ap(me_sb, g * GW_E + PAIR_E,
                        [(ME_FREE, 128), (ENEMY_F, EPG), (1, PAIR_E)]),
                    start=True, stop=True)
                nc.tensor.matmul(
                    pb[0:64, u * 128 : (u + 1) * 128], lhsT,
                    _ap(me_sb, g * GW_E,
                        [(ME_FREE, 128), (ENEMY_F, EPG), (1, PAIR_E)]),
                    start=True, stop=True)
            evac(
                _ap(s2e, g0 * EPG,
                    [(S2E_FREE, 128), (E_C, PAIR_E), (1, 4 * EPG)]),
                _ap(pb, 0,
                    [(512, 128), (1, PAIR_E), (128, 4), (PAIR_E, EPG)]))

        for b in range(NBATCH):
            a_stage(2 * b)
            a_stage(2 * b + 1)
            if b == 1:
                # w2 halves ride the scalar queue between prelus (needed
                # only by stage C, but must not sit behind the el DMAs)
                nc.scalar.dma_start(
                    w2_sb[:, 0 : W2COLS // 2], w2pack.ap()[:, 0 : W2COLS // 2])
                nc.scalar.dma_start(
                    w2_sb[:, W2COLS // 2 :], w2pack.ap()[:, W2COLS // 2 :])
            for j2 in range(BATCH // 2):
                b_ally(b * BATCH + 2 * j2)
            for j3 in range(BATCH // 4):
                b_enemy(b * BATCH + 4 * j3)

        # ---- stage C: out_T[o,e] accumulation over 40 fp slices ----
        pc = psC.tile([OUT, E_C], F32)
        n_sl = PAIR_A + PAIR_E
        idx = 0
        for fp in range(PAIR_A):
            nc.tensor.matmul(
                pc[:], w2_sb[:, fp * OUT : (fp + 1) * OUT],
                s2a[:, fp * E_C : (fp + 1) * E_C],
                start=(idx == 0), stop=(idx == n_sl - 1))
            idx += 1
        for fp in range(PAIR_E):
            nc.tensor.matmul(
                pc[:], w2_sb[:, (PAIR_A + fp) * OUT : (PAIR_A + fp + 1) * OUT],
                s2e[:, fp * E_C : (fp + 1) * E_C],
                start=(idx == 0), stop=(idx == n_sl - 1))
            idx += 1

        nc.vector.tensor_copy(osb[:], pc[:])
        nc.sync.dma_start(out_d.ap(), osb[:])


@functools.lru_cache(maxsize=2)
def _cached_program(alpha_a, alpha_e):
    return build_program(alpha_a, alpha_e)


def host_prep(ally_features, enemy_features, Wa1, ba1, aa, Wa2, ba2,
              We1, be1, ae, We2, be2):
    """Per-core input maps (numpy, bf16) + host-side bias term."""
    bf = ml_dtypes.bfloat16

    def uniform_alpha(a):
        a = np.asarray(a, dtype=np.float32)
        assert np.allclose(a, a[0]), "per-channel alpha not supported"
        return float(a[0])

    ua, ue = uniform_alpha(aa), uniform_alpha(ae)

    w1 = np.zeros((KROWS, 128), dtype=np.float32)
    w1[0:ALLY_F, 0:HYPER] = np.asarray(Wa1)
    w1[ALLY_F:80, HYPER:128] = np.asarray(We1)
    w1[80, 0:HYPER] = np.asarray(ba1)
    w1[80, HYPER:128] = np.asarray(be1)
    w1 = w1.astype(bf)

    w2 = np.zeros((128, W2COLS), dtype=np.float32)
    Wa2_, We2_ = np.asarray(Wa2), np.asarray(We2)
    for fp in range(PAIR_A):
        w2[0:HYPER, fp * OUT : (fp + 1) * OUT] = Wa2_[:, fp * OUT : (fp + 1) * OUT]
        w2[HYPER:128, fp * OUT : (fp + 1) * OUT] = \
            Wa2_[:, (fp + PAIR_A) * OUT : (fp + PAIR_A + 1) * OUT]
    for fp in range(PAIR_E):
        c = (PAIR_A + fp) * OUT
        w2[0:HYPER, c : c + OUT] = We2_[:, fp * OUT : (fp + 1) * OUT]
        w2[HYPER:128, c : c + OUT] = \
            We2_[:, (fp + PAIR_E) * OUT : (fp + PAIR_E + 1) * OUT]
    w2 = w2.astype(bf)

    fa_all = np.asarray(ally_features, dtype=np.float32)
    fe_all = np.asarray(enemy_features, dtype=np.float32)
    fa_bf = fa_all.astype(bf)
    fe_bf = fe_all.astype(bf)

    # host-side bias term: fsum @ b2 (exact fp32)
    fsum_a = fa_all.reshape(B_FULL, N_AGENTS, ALLY_F).sum(axis=1)
    fsum_e = fe_all.reshape(B_FULL, N_ENEMIES, ENEMY_F).sum(axis=1)
    bias_out = (fsum_a @ np.asarray(ba2).reshape(ALLY_F, OUT)
                + fsum_e @ np.asarray(be2).reshape(ENEMY_F, OUT)).astype(np.float32)

    RA = E_C * N_AGENTS
    RE = E_C * N_ENEMIES
    in_maps = []
    for c in range(N_CORES):
        fa_c = fa_bf[c * RA : (c + 1) * RA].reshape(E_C, N_AGENTS, ALLY_F)
        fe_c = fe_bf[c * RE : (c + 1) * RE].reshape(E_C, N_ENEMIES, ENEMY_F)
        # padded row-major features [E_C*16, featf], pad rows zero
        fa_pad = np.zeros((E_C, PITCH, ALLY_F), dtype=bf)
        fa_pad[:, :N_AGENTS, :] = fa_c
        fe_pad = np.zeros((E_C, PITCH, ENEMY_F), dtype=bf)
        fe_pad[:, :N_ENEMIES, :] = fe_c
        fa_pad = fa_pad.reshape(PROWS, ALLY_F)
        fe_pad = fe_pad.reshape(PROWS, ENEMY_F)
        # fsp: [81, PROWS] = [ally F^T padded; enemy F^T padded; ones]
        fs = np.zeros((KROWS, PROWS), dtype=bf)
        fs[0:ALLY_F, :] = fa_pad.T
        fs[ALLY_F:80, :] = fe_pad.T
        fs[80, :] = np.float32(1.0)

        # dense block-diag M image for the first IMG_G groups
        def m_image(f_pad, featf, gw, cols):
            img = np.zeros((EPG, PITCH, IMG_G, EPG, featf), dtype=bf)
            f4 = f_pad[: IMG_G * 128].reshape(IMG_G, EPG, PITCH, featf)
            for el in range(EPG):
                img[el, :, :, el, :] = f4[:, el].transpose(1, 0, 2)
            return np.ascontiguousarray(img.reshape(128, cols))

        in_maps.append({
            "fsp": np.ascontiguousarray(fs),
            "fap": np.ascontiguousarray(fa_pad),
            "fep": np.ascontiguousarray(fe_pad),
            "ma_img": m_image(fa_pad, ALLY_F, GW_A, IMA_COLS),
            "me_img": m_image(fe_pad, ENEMY_F, GW_E, IME_COLS),
            "w1e": w1, "w2pack": w2,
        })
    aux = {"bias_out": bias_out, "ua": ua, "ue": ue}
    return in_maps, aux


def assemble_output(results, aux):
    outs = [np.asarray(r["out"], dtype=np.float32) for r in results]
    dev = np.concatenate([o.T for o in outs], axis=0)
    return dev + aux["bias_out"]


def kernel(**inputs) -> np.ndarray:
    in_maps, aux = host_prep(**inputs)
    nc = _cached_program(aux["ua"], aux["ue"])
    res = run_bass_kernel_spmd(nc, in_maps, core_ids=list(range(N_CORES)))
    return assemble_output(res.results, aux)


if __name__ == "__main__":
    build_program()
    print("built ok")
